# revision 1
# baseline (speedup 1.0000x reference)
"""Neighbourhood attention block (7x7 window) on 8 Trainium2 NeuronCores.

Full inputs -> full output. Sharding: core = b*4 + g owns batch b and query
rows 16g..16g+15 (all 6 heads). Each core gets a 24-row halo slice of x,
transposed to feature-major and laid out in column-major token order
(token = wc*24 + r, wc = padded column 0..71, r = local row 0..23) so that
every 16-col x 24-row key window is a contiguous 384-token run: key chunks
(128 keys) and query blocks (128 queries = 8 cols x 16 rows) are then plain
2D slices, as required for matmul stationary operands.

Softmax runs without max-subtraction (scores are O(1)): keys on partitions,
probs = exp(s/8) * mask01 (bf16); the denominator comes free from a
ones-column appended to V (PV output col 64 of each 65-col head slot);
reciprocal + normalization are per-partition ops on the token-major PV
output; attn is then PE-transposed to feature-major for the out-projection.
"""
import sys

sys.path.insert(0, "/opt/trn_rl_repo")

import numpy as np
import ml_dtypes

import concourse.bass as bass
import concourse.mybir as mybir
from concourse import bacc
from concourse.tile import TileContext
from concourse.bass_utils import run_bass_kernel_spmd
from concourse.bass import broadcast_tensor_aps

F32 = mybir.dt.float32
BF16 = mybir.dt.bfloat16
AF = mybir.ActivationFunctionType

D = 384
NH = 6
E = 64
NCORES = 8
TOK = 1728          # 72 padded cols x 24 rows, column-major
SCALE = 0.125       # 1/sqrt(64)


def emit(nc):
    xT = nc.dram_tensor("xT", [D, TOK], F32, kind="ExternalInput").ap()
    wqkvT = nc.dram_tensor("wqkvT", [D, 3 * D], F32, kind="ExternalInput").ap()
    woutT = nc.dram_tensor("woutT", [D, D], F32, kind="ExternalInput").ap()
    ident = nc.dram_tensor("ident", [128, 128], F32, kind="ExternalInput").ap()
    mask = nc.dram_tensor("mask", [128, 8 * 3 * 128], BF16, kind="ExternalInput").ap()
    out = nc.dram_tensor("out", [8, 128, D], F32, kind="ExternalOutput").ap()

    with TileContext(nc) as tc:
        with tc.tile_pool(name="persist", bufs=1) as pp:
            xT_sb = [pp.tile([128, TOK], F32, tag=f"xT{i}", name=f"xT{i}")
                     for i in range(3)]
            w1_sb = [pp.tile([128, 3 * D], F32, tag=f"w1{i}", name=f"w1{i}")
                     for i in range(3)]
            w2_sb = [pp.tile([128, D], F32, tag=f"w2{i}", name=f"w2{i}")
                     for i in range(3)]
            id_sb = pp.tile([128, 128], F32, tag="id", name="idsb")
            mk_sb = pp.tile([128, 8 * 3 * 128], BF16, tag="mk", name="mksb")
            qT_sb = pp.tile([128, 3 * 1024], F32, tag="qT", name="qTsb")
            kT_sb = [pp.tile([128, TOK], F32, tag=f"kT{i}", name=f"kT{i}")
                     for i in range(3)]
            v_sb = pp.tile([128, 24 * 390], BF16, tag="v", name="vsb")

            for i in range(3):
                nc.sync.dma_start(out=xT_sb[i][:], in_=xT[i * 128:(i + 1) * 128, :])
                nc.sync.dma_start(out=w1_sb[i][:], in_=wqkvT[i * 128:(i + 1) * 128, :])
                nc.sync.dma_start(out=w2_sb[i][:], in_=woutT[i * 128:(i + 1) * 128, :])
            nc.sync.dma_start(out=id_sb[:], in_=ident[:])
            nc.sync.dma_start(out=mk_sb[:], in_=mask[:])

            # ones-columns of v (col 64 of each 65-col head slot)
            vv = v_sb[:].rearrange("p (c h e) -> p c h e", h=NH, e=65)
            nc.gpsimd.memset(vv[:, :, :, 64:65], 1.0)

            # ---- qkv projections ----
            with tc.tile_pool(name="qkps", bufs=3, space="PSUM") as qkp:
                # q^T: owned tokens (cols 4..67, rows 3..18), col-major
                for f in range(3):
                    for t in range(2):
                        ps = qkp.tile([128, 512], F32, tag="qk", name="qkps")
                        for d in range(3):
                            xv = xT_sb[d][:].rearrange("p (w r) -> p w r", r=24)
                            nc.tensor.matmul(
                                ps[:],
                                lhsT=w1_sb[d][:, f * 128:(f + 1) * 128],
                                rhs=xv[:, 4 + 32 * t:4 + 32 * (t + 1), 3:19],
                                start=(d == 0), stop=(d == 2),
                            )
                        nc.vector.tensor_copy(
                            out=qT_sb[:, f * 1024 + t * 512:f * 1024 + (t + 1) * 512],
                            in_=ps[:])
                # k^T over all 1728 tokens (zero pads give k=0)
                for f in range(3):
                    for t in range(4):
                        w = 512 if t < 3 else 192
                        ps = qkp.tile([128, 512], F32, tag="qk", name="qkps")
                        for d in range(3):
                            nc.tensor.matmul(
                                ps[:, :w],
                                lhsT=w1_sb[d][:, 384 + f * 128:384 + (f + 1) * 128],
                                rhs=xT_sb[d][:, t * 512:t * 512 + w],
                                start=(d == 0), stop=(d == 2),
                            )
                        nc.vector.tensor_copy(
                            out=kT_sb[f][:, t * 512:t * 512 + w], in_=ps[:, :w])
                # v in key-chunk layout [128 keys, 6*65] per (bw, c)
                for ch in range(24):
                    bw, c = divmod(ch, 3)
                    k0 = 192 * bw + 128 * c
                    ps = qkp.tile([128, 384], F32, tag="vps", name="vps")
                    for d in range(3):
                        nc.tensor.matmul(
                            ps[:],
                            lhsT=xT_sb[d][:, k0:k0 + 128],
                            rhs=w1_sb[d][:, 768:1152],
                            start=(d == 0), stop=(d == 2),
                        )
                    nc.vector.tensor_copy(
                        out=vv[:, ch, :, 0:64],
                        in_=ps[:].rearrange("p (h e) -> p h e", e=64))

            # ---- attention + output projection ----
            with tc.tile_pool(name="spool", bufs=2, space="PSUM") as spool, \
                 tc.tile_pool(name="pvpool", bufs=1, space="PSUM") as pvpool, \
                 tc.tile_pool(name="trpool", bufs=1, space="PSUM") as trpool, \
                 tc.tile_pool(name="fpool", bufs=1, space="PSUM") as fpool, \
                 tc.tile_pool(name="work", bufs=6) as wp, \
                 tc.tile_pool(name="work2", bufs=2) as wp2:
                for bw in range(8):
                    e_tiles = []
                    for c in range(3):
                        e_sb = wp.tile([128, 768], BF16, tag="e", name="esb")
                        k0 = 192 * bw + 128 * c
                        m1 = mk_sb[:, (bw * 3 + c) * 128:(bw * 3 + c + 1) * 128]
                        m3 = m1.rearrange("p (o q) -> p o q", o=1)
                        for ph in range(3):
                            # pair tile: head-even -> bank 0 (cols 0:128),
                            # head-odd -> bank 1 (cols 512:640); one matmul
                            # group per bank (HW requirement)
                            sps = spool.tile([128, 1024], F32, tag="s",
                                             name="sps")
                            for par in range(2):
                                h = 2 * ph + par
                                nc.tensor.matmul(
                                    sps[:, par * 512:par * 512 + 128],
                                    lhsT=kT_sb[ph][par * 64:par * 64 + 64,
                                                   k0:k0 + 128],
                                    rhs=qT_sb[par * 64:par * 64 + 64,
                                              ph * 1024 + bw * 128:
                                              ph * 1024 + (bw + 1) * 128],
                                    start=True, stop=True,
                                    tile_position=(par * 64, 0),
                                )
                            sps3 = sps[:].rearrange(
                                "p (b q) -> p b q", q=512)[:, :, 0:128]
                            e3 = e_sb[:, ph * 256:(ph + 1) * 256].rearrange(
                                "p (b q) -> p b q", q=128)
                            nc.scalar.activation(out=e3, in_=sps3, func=AF.Exp,
                                                 scale=SCALE)
                            a, b = broadcast_tensor_aps(e3, m3)
                            nc.vector.tensor_mul(out=e3, in0=a, in1=b)
                        e_tiles.append(e_sb)
                    at = wp.tile([128, 384], F32, tag="at", name="atsb")
                    aTt = wp.tile([128, 384], F32, tag="aTt", name="aTt")
                    for ph in range(3):
                        pv = pvpool.tile([128, 1024], F32, tag="pv", name="pvps")
                        rc = wp.tile([128, 2], F32, tag="rc", name="rcsb")
                        for par in range(2):
                            h = 2 * ph + par
                            for c in range(3):
                                nc.tensor.matmul(
                                    pv[:, par * 512:par * 512 + 65],
                                    lhsT=e_tiles[c][:, (2 * ph) * 128 + par * 128:
                                                    (2 * ph) * 128 + (par + 1) * 128],
                                    rhs=v_sb[:, (bw * 3 + c) * 390 + h * 65:
                                             (bw * 3 + c) * 390 + (h + 1) * 65],
                                    start=(c == 0), stop=(c == 2),
                                )
                            nc.vector.reciprocal_approx_fast(
                                out=rc[:, par:par + 1],
                                in_=pv[:, par * 512 + 64:par * 512 + 65])
                        pv3 = pv[:].rearrange("p (b q) -> p b q", q=512)[:, :, 0:64]
                        rc3 = rc[:].rearrange("p (h o) -> p h o", o=1)
                        at3 = at[:, ph * 128:(ph + 1) * 128].rearrange(
                            "p (h e) -> p h e", e=64)
                        a, b = broadcast_tensor_aps(pv3, rc3)
                        nc.vector.tensor_mul(out=at3, in0=a, in1=b)
                    # transpose attn [128 q, 384 f] -> attnT tiles [128 f, 128 q]
                    for d3 in range(3):
                        trp = trpool.tile([128, 128], F32, tag="tr", name="trps")
                        nc.tensor.transpose(
                            out=trp[:], in_=at[:, d3 * 128:(d3 + 1) * 128],
                            identity=id_sb[:])
                        nc.scalar.copy(
                            out=aTt[:, d3 * 128:(d3 + 1) * 128], in_=trp[:])
                    fps = fpool.tile([128, 384], F32, tag="f", name="fps")
                    for d3 in range(3):
                        nc.tensor.matmul(
                            fps[:],
                            lhsT=aTt[:, d3 * 128:(d3 + 1) * 128],
                            rhs=w2_sb[d3][:],
                            start=(d3 == 0), stop=(d3 == 2),
                        )
                    ob = wp2.tile([128, 384], F32, tag="ob", name="obsb")
                    nc.scalar.copy(out=ob[:], in_=fps[:])
                    nc.sync.dma_start(out=out[bw], in_=ob[:])
    return nc


def full_neighbourhood_mask():
    """[4096, 4096] bool, True where key inside query's 7x7 clipped window."""
    hp = np.arange(64)
    sh = np.clip(hp - 3, 0, 57)
    hr = np.arange(64)
    rowv = (hr[None, :] >= sh[:, None]) & (hr[None, :] < (sh + 7)[:, None])
    m = rowv[:, None, :, None] & rowv[None, :, None, :]  # [qh, qw, kh, kw]
    return m.reshape(64 * 64, 64 * 64)


def core_mask_arr(g, fullmask):
    """bf16 [128, 8*3*128]: keys-on-partitions masks for row-group g.

    key index: window pos p = 128*c + ki, p = wl*24 + r (wl = key col
    - (8*bw - 4), r = local row); query index qi = qc*16 + qr.
    """
    out = np.zeros((8, 3, 128, 128), np.float32)
    qr = np.arange(16)
    qc = np.arange(8)
    for bw in range(8):
        p = np.arange(384)
        wl, r = p // 24, p % 24
        krow = 16 * g - 3 + r
        kcol = 8 * bw - 4 + wl
        kvalid = (krow >= 0) & (krow < 64) & (kcol >= 0) & (kcol < 64)
        ktok = np.clip(krow, 0, 63) * 64 + np.clip(kcol, 0, 63)
        qrow = 16 * g + qr
        qcol = 8 * bw + qc
        # qi = qc*16 + qr -> qc outer, qr inner
        qtok = (qrow[None, :] * 64 + qcol[:, None]).ravel()
        m = fullmask[qtok[None, :], ktok[:, None].astype(np.intp)]  # [384, 128]
        m = m & kvalid[:, None]
        out[bw] = m.reshape(3, 128, 128)
    return np.ascontiguousarray(
        out.transpose(2, 0, 1, 3).reshape(128, 8 * 3 * 128)
    ).astype(ml_dtypes.bfloat16)


_NC_CACHE = {}


def build():
    if "nc" not in _NC_CACHE:
        nc = bacc.Bacc("TRN2", target_bir_lowering=False, debug=False)
        emit(nc)
        nc.compile()
        _NC_CACHE["nc"] = nc
    return _NC_CACHE["nc"]


def make_in_maps(x, w_qkv, w_out):
    x = np.asarray(x, np.float32)
    wqkvT = np.ascontiguousarray(np.asarray(w_qkv, np.float32).T)
    woutT = np.ascontiguousarray(np.asarray(w_out, np.float32).T)
    ident = np.eye(128, dtype=np.float32)
    fullmask = full_neighbourhood_mask()
    gmasks = [core_mask_arr(g, fullmask) for g in range(4)]
    in_maps = []
    for core in range(NCORES):
        b, g = core // 4, core % 4
        rows = np.arange(16 * g - 3, 16 * g + 21)
        xs = np.zeros((24, 72, D), np.float32)  # [r, wc, D]
        valid = (rows >= 0) & (rows < 64)
        xs[valid, 4:68] = x[b, rows[valid]]
        # col-major tokens: token = wc*24 + r
        xT = np.ascontiguousarray(xs.transpose(2, 1, 0).reshape(D, 72 * 24))
        in_maps.append({
            "xT": xT, "wqkvT": wqkvT, "woutT": woutT,
            "ident": ident, "mask": gmasks[g],
        })
    return in_maps


def gather(results):
    full = np.zeros((2, 64, 64, D), np.float32)
    for core in range(NCORES):
        b, g = core // 4, core % 4
        o = results[core]["out"]  # [bw, qi = qc*16 + qr, f]
        o = o.reshape(8, 8, 16, D).transpose(2, 0, 1, 3).reshape(16, 64, D)
        full[b, 16 * g:16 * g + 16] = o
    return full


def kernel(x, w_qkv, w_out):
    nc = build()
    in_maps = make_in_maps(x, w_qkv, w_out)
    res = run_bass_kernel_spmd(nc, in_maps, core_ids=list(range(NCORES)))
    return gather(res.results)


def np_reference(x, w_qkv, w_out):
    """Plain-numpy port of reference.py for offline validation."""
    B, H, W, Dd = x.shape
    nh = Dd // E
    N = H * W
    qkv = x.reshape(B * N, Dd) @ w_qkv.T
    qkv = qkv.reshape(B, N, 3, nh, E).transpose(2, 0, 3, 1, 4)
    q, k, v = qkv[0], qkv[1], qkv[2]
    m = full_neighbourhood_mask()
    s = np.einsum("bnqe,bnke->bnqk", q, k) * (1.0 / np.sqrt(E))
    s = np.where(m[None, None], s, -np.inf)
    s = s - s.max(-1, keepdims=True)
    p = np.exp(s)
    p /= p.sum(-1, keepdims=True)
    o = np.einsum("bnqk,bnke->bnqe", p, v)
    o = o.transpose(0, 2, 1, 3).reshape(B, H, W, Dd)
    return o @ w_out.T


if __name__ == "__main__":
    from concourse.bass_interp import CoreSim
    rng = np.random.default_rng(0)
    x = rng.standard_normal((2, 64, 64, D), dtype=np.float32)
    w_qkv = (rng.standard_normal((3 * D, D)) * 0.02).astype(np.float32)
    w_out = (rng.standard_normal((D, D)) * 0.02).astype(np.float32)
    expected = np_reference(x, w_qkv, w_out)
    nc = build()
    in_maps = make_in_maps(x, w_qkv, w_out)
    core = int(sys.argv[1]) if len(sys.argv) > 1 else 0
    sim = CoreSim(nc)
    for kk, v in in_maps[core].items():
        sim.tensor(kk)[:] = v
    sim.simulate()
    got = np.array(sim.tensor("out"))
    b, g = core // 4, core % 4
    got = got.reshape(8, 8, 16, D).transpose(2, 0, 1, 3).reshape(16, 64, D)
    exp = expected[b, 16 * g:16 * g + 16]
    rel = np.linalg.norm(got - exp) / np.linalg.norm(exp)
    print(f"core {core}: rel_l2={rel:.3e} "
          f"absmax_rel={np.abs(got - exp).max() / np.abs(exp).max():.3e}")



# revision 43
# speedup vs baseline: 3.6111x; 3.6111x over previous
"""Neighbourhood attention block (7x7 window) on 8 Trainium2 NeuronCores.

Full inputs -> full output. Sharding: core = b*4 + g owns batch b and query
rows 16g..16g+15 (all 6 heads). Each core gets a 24-row halo slice of x,
transposed to feature-major and laid out in column-major token order
(token = wc*24 + r, wc = padded column 0..71, r = local row 0..23) so that
every 16-col x 24-row key window is a contiguous 384-token run.

All matmul operands are fp16 (1 cycle/row on the PE vs 4 for fp32);
accumulation stays fp32 in PSUM. V is projected once per global 128-token
chunk; odd window blocks read a 64-token-shifted SBUF copy of V so every PV
window is 3 aligned chunks. Consecutive matmul accumulation groups must
land in different PSUM banks (hardware requirement), so per-tile head
layouts alternate banks: head h sits at column (h%2)*bank + (h//2)*slot.

Softmax runs without max-subtraction (scores are O(1)): keys on partitions,
probs = exp(s/8) * mask01 (fp16); the denominator comes free from a
ones-column appended to V (PV output col 64 of each 65-col head slot);
reciprocal + normalization are per-partition ops on the token-major PV
output; attn is then PE-transposed to feature-major for the out-projection.
"""
import sys

sys.path.insert(0, "/opt/trn_rl_repo")

import numpy as np

import concourse.bass as bass
import concourse.mybir as mybir
from concourse import bacc
from concourse.tile import TileContext
from concourse.bass_utils import run_bass_kernel_spmd
from concourse.bass import broadcast_tensor_aps

F32 = mybir.dt.float32
F16 = mybir.dt.float16
AF = mybir.ActivationFunctionType

D = 384
NH = 6
E = 64
NCORES = 8
TOK = 1728          # 72 padded cols x 24 rows, column-major
SCALE = 0.125       # 1/sqrt(64)


def emit(nc):
    xT = nc.dram_tensor("xT", [D, TOK], F16, kind="ExternalInput").ap()
    wqkvT = nc.dram_tensor("wqkvT", [D, 3 * D], F16, kind="ExternalInput").ap()
    woutT = nc.dram_tensor("woutT", [D, D], F16, kind="ExternalInput").ap()
    ident = nc.dram_tensor("ident", [128, 128], F16, kind="ExternalInput").ap()
    mask = nc.dram_tensor("mask", [128, 8 * 3 * 128], F16, kind="ExternalInput").ap()
    out = nc.dram_tensor("out", [8, 128, D], F16, kind="ExternalOutput").ap()

    with TileContext(nc) as tc:
        with tc.tile_pool(name="persist", bufs=1) as pp, \
             tc.tile_pool(name="spool", bufs=3, space="PSUM") as spool:
            xT_sb = [pp.tile([128, TOK], F16, tag=f"xT{i}", name=f"xT{i}")
                     for i in range(3)]
            w1_sb = [pp.tile([128, 3 * D], F16, tag=f"w1{i}", name=f"w1{i}")
                     for i in range(3)]
            w2_sb = [pp.tile([128, D], F16, tag=f"w2{i}", name=f"w2{i}")
                     for i in range(3)]
            id_sb = pp.tile([128, 128], F16, tag="id", name="idsb")
            mk_sb = pp.tile([128, 8 * 3 * 128], F16, tag="mk", name="mksb")
            qT_sb = pp.tile([128, 3 * 1024], F16, tag="qT", name="qTsb")
            kT_sb = [pp.tile([128, TOK], F16, tag=f"kT{i}", name=f"kT{i}")
                     for i in range(3)]
            v_sb = pp.tile([128, 13 * 390], F16, tag="v", name="vsb")
            # 64-token-shifted copy of v: slot m holds tokens
            # 192+128m..320+128m, so odd-bw PV windows are 3 aligned chunks
            v2_sb = pp.tile([128, 12 * 390], F16, tag="v2", name="v2sb")
            dummy = pp.tile([128, 256], F16, tag="dum", name="dummysb")
            scratch = pp.tile([128, 128], F16, tag="scr", name="scratchsb")

            # ---- input DMAs, split across the SP and Act HWDGE queues ----
            # xT pieces align with the k-projection chunk boundaries
            # (96+512t) so each chunk's operands arrive as early as possible
            for d in range(3):
                nc.sync.dma_start(out=xT_sb[d][:, 0:608],
                                  in_=xT[d * 128:(d + 1) * 128, 0:608])
            for d in range(3):
                nc.scalar.dma_start(out=w1_sb[d][:, 384:768],
                                    in_=wqkvT[d * 128:(d + 1) * 128, 384:768])
            for d in range(3):
                nc.scalar.dma_start(out=xT_sb[d][:, 608:1120],
                                    in_=xT[d * 128:(d + 1) * 128, 608:1120])
            for d in range(3):
                nc.sync.dma_start(out=w1_sb[d][:, 0:384],
                                  in_=wqkvT[d * 128:(d + 1) * 128, 0:384])
            nc.scalar.dma_start(out=id_sb[:], in_=ident[:])
            for d in range(3):
                nc.sync.dma_start(out=xT_sb[d][:, 1120:1728],
                                  in_=xT[d * 128:(d + 1) * 128, 1120:1728])
            for d in range(3):
                nc.scalar.dma_start(out=w1_sb[d][:, 768:1152],
                                    in_=wqkvT[d * 128:(d + 1) * 128, 768:1152])
            nc.sync.dma_start(out=mk_sb[:, 0:1536], in_=mask[:, 0:1536])
            nc.sync.dma_start(out=mk_sb[:, 1536:3072], in_=mask[:, 1536:3072])
            for d in range(3):
                nc.sync.dma_start(out=w2_sb[d][:],
                                  in_=woutT[d * 128:(d + 1) * 128, :])
            # Pool engine: memsets (it cannot touch PSUM)
            nc.gpsimd.memset(dummy[:], 0.0)
            vv = v_sb[:].rearrange("p (c h e) -> p c h e", h=NH, e=65)
            nc.gpsimd.memset(vv[:, :, :, 64:65], 1.0)
            for f in range(3):
                nc.gpsimd.memset(kT_sb[f][:, 0:96], 0.0)
                nc.gpsimd.memset(kT_sb[f][:, 1632:1728], 0.0)
            # Act: exp-table preload once ident is in
            nc.scalar.activation(out=scratch[:], in_=id_sb[:],
                                 func=AF.Exp, scale=1.0)

            # ---- software-pipelined main loop ----
            # Stages per iteration i: scores/exp/mask for bw=i, lazy v
            # chunks for bw=i, transpose for bw=i-2, PV+normalize for
            # bw=i-1, out-projection for bw=i-2. The stage skew keeps the
            # PE from stalling on the Act/DVE chains of the same bw. q/k
            # projection chains are folded into iterations 0-2.
            with tc.tile_pool(name="warm", bufs=2, space="PSUM") as wmp:
                # keep the PE busy through the DMA fill so the p-state ramp
                # reaches full clock before the real matmuls begin; alternate
                # two tiles so consecutive groups hit different banks
                wA = wmp.tile([128, 256], F32, tag="wm", name="wmpsA")
                wB = wmp.tile([128, 256], F32, tag="wm", name="wmpsB")
                for i in range(9):
                    nc.tensor.matmul((wA if i % 2 else wB)[:],
                                     lhsT=dummy[:, 0:128],
                                     rhs=dummy[:], start=True, stop=True)
                for i in range(14):
                    nc.tensor.matmul((wA if i % 2 else wB)[:, 0:64],
                                     lhsT=dummy[:, 0:128],
                                     rhs=dummy[:, 0:64], start=True, stop=True)
            with tc.tile_pool(name="upool", bufs=2, space="PSUM") as ulp, \
                 tc.tile_pool(name="work", bufs=9) as wp, \
                 tc.tile_pool(name="work2", bufs=3) as wp2:

                # PSUM can only be read by the DVE and Act engines; alternate
                # the projection copies between them
                cp_toggle = [0]

                def psum_copy(out_, in_):
                    cp_toggle[0] += 1
                    if cp_toggle[0] <= 6 or cp_toggle[0] % 2:
                        nc.vector.tensor_copy(out=out_, in_=in_)
                    else:
                        nc.scalar.copy(out=out_, in_=in_)

                def q_chain(f, t):
                    ps = ulp.tile([128, 512], F32, tag="u", name="qkps")
                    for d in range(3):
                        xv = xT_sb[d][:].rearrange("p (w r) -> p w r", r=24)
                        nc.tensor.matmul(
                            ps[:],
                            lhsT=w1_sb[d][:, f * 128:(f + 1) * 128],
                            rhs=xv[:, 4 + 32 * t:4 + 32 * (t + 1), 3:19],
                            start=(d == 0), stop=(d == 2),
                        )
                    psum_copy(
                        qT_sb[:, f * 1024 + t * 512:f * 1024 + (t + 1) * 512],
                        ps[:])

                def k_chain(f, t):
                    # real tokens 96..1632 in three 512-wide chunks; pads of
                    # kT are memset to zero (exp(0)=1 then masked off)
                    c0 = 96 + t * 512
                    ps = ulp.tile([128, 512], F32, tag="u", name="qkps")
                    for d in range(3):
                        nc.tensor.matmul(
                            ps[:],
                            lhsT=w1_sb[d][:, 384 + f * 128:384 + (f + 1) * 128],
                            rhs=xT_sb[d][:, c0:c0 + 512],
                            start=(d == 0), stop=(d == 2),
                        )
                    psum_copy(kT_sb[f][:, c0:c0 + 512], ps[:])

                def v_chunk(j):
                    # chunk 13 is all padding (zero); chunks run 0..12 only
                    ps = ulp.tile([128, 512], F32, tag="u", name="vps")
                    for d in range(3):
                        nc.tensor.matmul(
                            ps[:, 0:384],
                            lhsT=xT_sb[d][:, 128 * j:128 * j + 128],
                            rhs=w1_sb[d][:, 768:1152],
                            start=(d == 0), stop=(d == 2),
                        )
                    nc.vector.tensor_copy(
                        out=vv[:, j, :, 0:64],
                        in_=ps[:, 0:384].rearrange("p (h e) -> p h e", e=64))
                    # shifted copy slot j-2 (= tokens 192+128(j-2) ..) becomes
                    # complete once chunk j is in SBUF; SBUF-to-SBUF, so the
                    # otherwise-idle GPSIMD engine does it
                    if j >= 2:
                        m = j - 2
                        nc.gpsimd.tensor_copy(
                            out=v2_sb[0:64, m * 390:(m + 1) * 390],
                            in_=v_sb[64:128, (j - 1) * 390:j * 390])
                        nc.gpsimd.tensor_copy(
                            out=v2_sb[64:128, m * 390:(m + 1) * 390],
                            in_=v_sb[0:64, j * 390:(j + 1) * 390])
                    if j == 12:
                        nc.gpsimd.tensor_copy(
                            out=v2_sb[0:64, 11 * 390:12 * 390],
                            in_=v_sb[64:128, 12 * 390:13 * 390])
                        nc.gpsimd.memset(
                            v2_sb[64:128, 11 * 390:12 * 390], 0.0)

                def stage_s(bw):
                    # head h scores at psum col (h%2)*512 + (h//2)*128 so
                    # consecutive groups alternate banks; e_sb keeps the same
                    # parity-major layout at (h%2)*384 + (h//2)*128
                    t0 = 192 * bw
                    e_tiles = []
                    for c in range(3):
                        k0 = t0 + 128 * c
                        sps = spool.tile([128, 1024], F32, tag="s", name="sps")
                        for h in range(6):
                            ph, par = h // 2, h % 2
                            sc = par * 512 + ph * 128
                            nc.tensor.matmul(
                                sps[:, sc:sc + 128],
                                lhsT=kT_sb[ph][par * 64:par * 64 + 64,
                                               k0:k0 + 128],
                                rhs=qT_sb[par * 64:par * 64 + 64,
                                          ph * 1024 + bw * 128:
                                          ph * 1024 + (bw + 1) * 128],
                                start=True, stop=True,
                            )
                        e_sb = wp.tile([128, 768], F16, tag="e", name="esb")
                        s4 = sps[:].rearrange("p (par i q) -> p par i q",
                                              par=2, q=128)[:, :, 0:3]
                        e4 = e_sb[:].rearrange("p (par i q) -> p par i q",
                                               par=2, q=128)
                        nc.scalar.activation(out=e4, in_=s4,
                                             func=AF.Exp, scale=SCALE)
                        m1 = mk_sb[:, (bw * 3 + c) * 128:(bw * 3 + c + 1) * 128]
                        m4 = m1.rearrange("p (a b q) -> p a b q", a=1, b=1)
                        a, b = broadcast_tensor_aps(e4, m4)
                        eng = nc.gpsimd if c == 0 else nc.vector
                        eng.tensor_mul(out=e4, in0=a, in1=b)
                        e_tiles.append(e_sb)
                    return e_tiles

                def ecol(h):
                    return (h % 2) * 384 + (h // 2) * 128

                def stage_p(bw, e_tiles):
                    # head h at pv col (h%2)*512 + (h//2)*65: consecutive
                    # accumulation chains alternate banks
                    t0 = 192 * bw
                    if bw % 2 == 0:
                        vsrc, j0 = v_sb, t0 // 128
                    else:
                        vsrc, j0 = v2_sb, 3 * (bw - 1) // 2
                    pv = spool.tile([128, 1024], F32, tag="s", name="pvps")
                    for h in range(6):
                        pc = (h % 2) * 512 + (h // 2) * 65
                        for c in range(3):
                            j = j0 + c
                            nc.tensor.matmul(
                                pv[:, pc:pc + 65],
                                lhsT=e_tiles[c][:, ecol(h):ecol(h) + 128],
                                rhs=vsrc[:, j * 390 + h * 65:
                                         j * 390 + (h + 1) * 65],
                                start=(c == 0), stop=(c == 2),
                            )
                    pvh = pv[:].rearrange("p (par r) -> p par r", par=2)
                    pvh = pvh[:, :, 0:195].rearrange(
                        "p par (i x) -> p par i x", x=65)
                    rc = wp.tile([128, 8], F32, tag="rc", name="rcsb", bufs=3)
                    for par in range(2):
                        pvd = pv[:, par * 512:par * 512 + 195].rearrange(
                            "p (i x) -> p i x", x=65)[:, :, 64:65]
                        rcd = rc[:, 3 * par:3 * par + 3].rearrange(
                            "p (i o) -> p i o", o=1)
                        nc.vector.reciprocal_approx_fast(out=rcd, in_=pvd)
                    rc3 = rc[:, 0:6].rearrange("p (par i o) -> p par i o",
                                               par=2, o=1)
                    at = wp.tile([128, 384], F16, tag="at", name="atsb", bufs=3)
                    # natural head-major storage via stride permutation:
                    # col(h=2i+par, e) = (2i+par)*64 + e
                    at4 = at[:].rearrange("p (i par e) -> p par i e",
                                          par=2, e=64)
                    a, b = broadcast_tensor_aps(pvh[:, :, :, 0:64], rc3)
                    nc.vector.tensor_mul(out=at4, in0=a, in1=b)
                    return at

                def stage_tr(at):
                    # transpose attn [128 q, 384 f] -> attnT [128 f, 128 q]
                    # x3; two psum tiles so consecutive groups alternate banks
                    trpA = ulp.tile([128, 512], F16, tag="u", name="trpA")
                    trpB = ulp.tile([128, 512], F16, tag="u", name="trpB")
                    nc.tensor.transpose(out=trpA[:, 0:128],
                                        in_=at[:, 0:128], identity=id_sb[:])
                    nc.tensor.transpose(out=trpB[:, 0:128],
                                        in_=at[:, 128:256], identity=id_sb[:])
                    nc.tensor.transpose(out=trpA[:, 128:256],
                                        in_=at[:, 256:384], identity=id_sb[:])
                    aTt = wp.tile([128, 384], F16, tag="aTt", name="aTt",
                                  bufs=3)
                    nc.vector.tensor_copy(out=aTt[:, 0:256],
                                          in_=trpA[:, 0:256])
                    nc.vector.tensor_copy(out=aTt[:, 256:384],
                                          in_=trpB[:, 0:128])
                    return aTt

                def stage_op(bw, aTt):
                    # aTt holds f-tiles in order d3 = 0, 2, 1 (transpose
                    # placement); contract against the matching w2 tiles
                    if bw < 7:
                        fps = ulp.tile([128, 512], F32, tag="u", name="fps")
                        for pos, d3 in enumerate((0, 2, 1)):
                            nc.tensor.matmul(
                                fps[:, 0:384],
                                lhsT=aTt[:, pos * 128:(pos + 1) * 128],
                                rhs=w2_sb[d3][:],
                                start=(pos == 0), stop=(pos == 2),
                            )
                        ob = wp2.tile([128, 384], F16, tag="ob", name="obsb")
                        nc.scalar.copy(out=ob[:], in_=fps[:, 0:384])
                        nc.sync.dma_start(out=out[bw], in_=ob[:])
                        return
                    # last block: two output-column halves so the final
                    # ob-copy + DMA latency chains overlap
                    fpa = ulp.tile([128, 512], F32, tag="u", name="fpa")
                    fpb = ulp.tile([128, 512], F32, tag="u", name="fpb")
                    ob = wp2.tile([128, 384], F16, tag="ob", name="obsb")
                    for half, fp in ((0, fpa), (1, fpb)):
                        cl, cr = half * 192, half * 192 + 192
                        for pos, d3 in enumerate((0, 2, 1)):
                            nc.tensor.matmul(
                                fp[:, cl:cr],
                                lhsT=aTt[:, pos * 128:(pos + 1) * 128],
                                rhs=w2_sb[d3][:, cl:cr],
                                start=(pos == 0), stop=(pos == 2),
                            )
                        if half == 0:
                            nc.scalar.copy(out=ob[:, cl:cr], in_=fp[:, cl:cr])
                        else:
                            nc.vector.tensor_copy(out=ob[:, cl:cr],
                                                  in_=fp[:, cl:cr])
                        eng = nc.sync if half == 0 else nc.scalar
                        eng.dma_start(out=out[bw][:, cl:cr], in_=ob[:, cl:cr])

                v_done = 0
                e_ctx = {}
                at_ctx = {}
                aTt_ctx = {}
                for i in range(10):
                    if i == 0:
                        for f in range(3):
                            k_chain(f, 0)
                        for f in range(3):
                            q_chain(f, 0)
                    elif i == 1:
                        for f in range(3):
                            k_chain(f, 1)
                        for f in range(3):
                            q_chain(f, 1)
                    elif i == 2:
                        for f in range(3):
                            k_chain(f, 2)
                    if i < 8:
                        e_ctx[i] = stage_s(i)
                        jmax = min((192 * i + 383) // 128, 12)
                        while v_done <= jmax:
                            v_chunk(v_done)
                            v_done += 1
                    if i >= 2:
                        aTt_ctx[i - 2] = stage_tr(at_ctx.pop(i - 2))
                    if 1 <= i <= 8:
                        at_ctx[i - 1] = stage_p(i - 1, e_ctx.pop(i - 1))
                    if i >= 2:
                        stage_op(i - 2, aTt_ctx.pop(i - 2))
    return nc


def full_neighbourhood_mask():
    """[4096, 4096] bool, True where key inside query's 7x7 clipped window."""
    hp = np.arange(64)
    sh = np.clip(hp - 3, 0, 57)
    hr = np.arange(64)
    rowv = (hr[None, :] >= sh[:, None]) & (hr[None, :] < (sh + 7)[:, None])
    m = rowv[:, None, :, None] & rowv[None, :, None, :]  # [qh, qw, kh, kw]
    return m.reshape(64 * 64, 64 * 64)


def core_mask_arr(g, fullmask):
    """fp16 [128, 8*3*128]: keys-on-partitions masks for row-group g.

    key index: window pos p = 128*c + ki, p = wl*24 + r (wl = key col
    - (8*bw - 4), r = local row); query index qi = qc*16 + qr.
    """
    out = np.zeros((8, 3, 128, 128), np.float32)
    qr = np.arange(16)
    qc = np.arange(8)
    for bw in range(8):
        p = np.arange(384)
        wl, r = p // 24, p % 24
        krow = 16 * g - 3 + r
        kcol = 8 * bw - 4 + wl
        kvalid = (krow >= 0) & (krow < 64) & (kcol >= 0) & (kcol < 64)
        ktok = np.clip(krow, 0, 63) * 64 + np.clip(kcol, 0, 63)
        qrow = 16 * g + qr
        qcol = 8 * bw + qc
        # qi = qc*16 + qr -> qc outer, qr inner
        qtok = (qrow[None, :] * 64 + qcol[:, None]).ravel()
        m = fullmask[qtok[None, :], ktok[:, None].astype(np.intp)]  # [384, 128]
        m = m & kvalid[:, None]
        out[bw] = m.reshape(3, 128, 128)
    return np.ascontiguousarray(
        out.transpose(2, 0, 1, 3).reshape(128, 8 * 3 * 128)
    ).astype(np.float16)


_NC_CACHE = {}


def build():
    if "nc" not in _NC_CACHE:
        nc = bacc.Bacc("TRN2", target_bir_lowering=False, debug=False)
        emit(nc)
        nc.compile()
        _NC_CACHE["nc"] = nc
    return _NC_CACHE["nc"]


def make_in_maps(x, w_qkv, w_out):
    x = np.asarray(x, np.float32)
    wqkvT = np.ascontiguousarray(np.asarray(w_qkv, np.float32).T).astype(np.float16)
    woutT = np.ascontiguousarray(np.asarray(w_out, np.float32).T).astype(np.float16)
    ident = np.eye(128, dtype=np.float16)
    fullmask = full_neighbourhood_mask()
    gmasks = [core_mask_arr(g, fullmask) for g in range(4)]
    in_maps = []
    for core in range(NCORES):
        b, g = core // 4, core % 4
        rows = np.arange(16 * g - 3, 16 * g + 21)
        xs = np.zeros((24, 72, D), np.float32)  # [r, wc, D]
        valid = (rows >= 0) & (rows < 64)
        xs[valid, 4:68] = x[b, rows[valid]]
        # col-major tokens: token = wc*24 + r
        xT = np.ascontiguousarray(
            xs.transpose(2, 1, 0).reshape(D, 72 * 24)).astype(np.float16)
        in_maps.append({
            "xT": xT, "wqkvT": wqkvT, "woutT": woutT,
            "ident": ident, "mask": gmasks[g],
        })
    return in_maps


def gather(results):
    full = np.zeros((2, 64, 64, D), np.float32)
    for core in range(NCORES):
        b, g = core // 4, core % 4
        o = results[core]["out"]  # [bw, qi = qc*16 + qr, f]
        o = np.asarray(o, np.float32)
        o = o.reshape(8, 8, 16, D).transpose(2, 0, 1, 3).reshape(16, 64, D)
        full[b, 16 * g:16 * g + 16] = o
    return full


def kernel(x, w_qkv, w_out):
    nc = build()
    in_maps = make_in_maps(x, w_qkv, w_out)
    res = run_bass_kernel_spmd(nc, in_maps, core_ids=list(range(NCORES)))
    return gather(res.results)


def np_reference(x, w_qkv, w_out):
    """Plain-numpy port of reference.py for offline validation."""
    B, H, W, Dd = x.shape
    nh = Dd // E
    N = H * W
    qkv = x.reshape(B * N, Dd) @ w_qkv.T
    qkv = qkv.reshape(B, N, 3, nh, E).transpose(2, 0, 3, 1, 4)
    q, k, v = qkv[0], qkv[1], qkv[2]
    m = full_neighbourhood_mask()
    s = np.einsum("bnqe,bnke->bnqk", q, k) * (1.0 / np.sqrt(E))
    s = np.where(m[None, None], s, -np.inf)
    s = s - s.max(-1, keepdims=True)
    p = np.exp(s)
    p /= p.sum(-1, keepdims=True)
    o = np.einsum("bnqk,bnke->bnqe", p, v)
    o = o.transpose(0, 2, 1, 3).reshape(B, H, W, Dd)
    return o @ w_out.T


if __name__ == "__main__":
    from concourse.bass_interp import CoreSim
    rng = np.random.default_rng(0)
    x = rng.standard_normal((2, 64, 64, D), dtype=np.float32)
    w_qkv = (rng.standard_normal((3 * D, D)) * 0.02).astype(np.float32)
    w_out = (rng.standard_normal((D, D)) * 0.02).astype(np.float32)
    expected = np_reference(x, w_qkv, w_out)
    nc = build()
    in_maps = make_in_maps(x, w_qkv, w_out)
    core = int(sys.argv[1]) if len(sys.argv) > 1 else 0
    sim = CoreSim(nc)
    for kk, v in in_maps[core].items():
        sim.tensor(kk)[:] = v
    sim.simulate()
    got = np.array(sim.tensor("out"))
    b, g = core // 4, core % 4
    got = got.reshape(8, 8, 16, D).transpose(2, 0, 1, 3).reshape(16, 64, D)
    exp = expected[b, 16 * g:16 * g + 16]
    rel = np.linalg.norm(got - exp) / np.linalg.norm(exp)
    print(f"core {core}: rel_l2={rel:.3e} "
          f"absmax_rel={np.abs(got - exp).max() / np.abs(exp).max():.3e}")
    print(f"sim time: {sim.time:.0f} ns")
